# revision 1
# baseline (speedup 1.0000x reference)
"""Trainium2 Bass kernel for nn_CandidateFinder (LSH hash-equality KNN).

Reference semantics: q/k binarized (x>0), projected by W [64,8], sign bits
packed into an 8-bit bucket code; for each query, return the first 64 key
indices (ascending) whose code equals the query's code, padded with -1.

Key insight: codes live in [0,256). Build, per batch, a [256, 64] table of
the first 64 key indices per bucket, then gather per query. Both steps map
onto matmuls + a free-dim prefix scan + one GPSIMD local_scatter.

Sharding: 8 cores = 4 batches x 2 bucket-halves (c in [0,128) / [128,256)).
Each core computes a partial gather (zero where the query's code is in the
other half); host sums the pair and subtracts 1 (table stores j+1, empty=0).

Precision: the hash sign test needs ~f32-accurate projections. W is split
as fp16(W) + fp16(W - fp16(W)) and the two fp16 matmuls accumulate in f32
PSUM; representation error ~1e-6 vs hash sign margins ~1e-4 on this data.
"""

import numpy as np
import ml_dtypes

B, L, D, NH = 4, 2048, 64, 8
KMAX = 64
TABLE_ELEMS = 1024  # local_scatter num_elems; must exceed max bucket count
MPAD = 40           # hash matmul lhsT free size: 8 real + 32 zero rows
HALF = L // 2

_cache = {}


def _build_program():
    import concourse.bass as bass
    import concourse.mybir as mybir
    from concourse import bacc, tile
    from contextlib import ExitStack

    dt = mybir.dt
    Alu = mybir.AluOpType
    Act = mybir.ActivationFunctionType

    nc = bacc.Bacc("TRN2", target_bir_lowering=False, debug=False)

    # DRAM I/O (per-core shapes)
    qT_d = nc.declare_dram_parameter("qT", [D, L], dt.bfloat16, isOutput=False)
    kT_d = nc.declare_dram_parameter("kT", [D, L], dt.bfloat16, isOutput=False)
    # packed [Whi | Wlo] fp16, each [64, MPAD] (cols 8..MPAD zero)
    wpk_d = nc.declare_dram_parameter("wpk", [D, 2 * MPAD], dt.float16, isOutput=False)
    sgnc_d = nc.declare_dram_parameter("sgnc", [MPAD, 128], dt.float16, isOutput=False)
    out_d = nc.declare_dram_parameter("out", [L, KMAX], dt.float16, isOutput=True)

    with ExitStack() as ctx:
        tc = ctx.enter_context(tile.TileContext(nc))
        sb = ctx.enter_context(tc.tile_pool(name="sb", bufs=1))
        ps = ctx.enter_context(tc.tile_pool(name="ps", bufs=3, space="PSUM"))
        psw = ctx.enter_context(tc.tile_pool(name="psw", bufs=1, space="PSUM"))
        aps = ctx.enter_context(tc.tile_pool(name="aps", bufs=2, space="PSUM"))

        # ---- loads ----
        # weights on the DVE hwdge queue; big loads split across sync + ACT
        # queues so each tensor lands in ~half the single-queue time.
        kT_sb = sb.tile([D, L], dt.bfloat16, tag="kT")
        nc.sync.dma_start(kT_sb[:, 0:HALF], kT_d[:, 0:HALF])
        nc.gpsimd.dma_start(kT_sb[:, HALF:L], kT_d[:, HALF:L])
        wpk_sb = sb.tile([D, 2 * MPAD], dt.float16, tag="wpk")
        nc.gpsimd.dma_start(wpk_sb[:], wpk_d[:])
        qT_sb = sb.tile([D, L], dt.bfloat16, tag="qT")
        nc.sync.dma_start(qT_sb[:, 0:HALF], qT_d[:, 0:HALF])
        nc.gpsimd.dma_start(qT_sb[:, HALF:L], qT_d[:, HALF:L])
        sgnc_sb = sb.tile([MPAD, 128], dt.float16, tag="sgnc")
        nc.sync.dma_start(sgnc_sb[:], sgnc_d[:])

        # iota data for the scatter: each partition holds 1..L (int16)
        iota_sb = sb.tile([128, L], dt.int16, tag="iota")
        nc.gpsimd.iota(iota_sb[:], pattern=[[1, L]], base=1, channel_multiplier=0)

        # bias constant for the one-hot Relu(agree - 7)
        bias7 = sb.tile([128, 1], dt.float32, tag="bias7")
        nc.gpsimd.memset(bias7[:], -7.0)

        # ---- PE warm-up: keep the tensor engine busy from t~1us so the
        # p-state ramp completes before the real matmuls arrive ----
        warm_src = sb.tile([D, 512], dt.float16, tag="warm")
        nc.vector.memset(warm_src[:], 0.0)
        wp = psw.tile([MPAD, 512], dt.float32, tag="warmp")
        for r in range(6):
            nc.tensor.matmul(
                wp[:], lhsT=warm_src[:, 0:MPAD], rhs=warm_src[:],
                start=True, stop=True,
            )
        warm_sink = sb.tile([1, 1], dt.float32, tag="warmsink")
        nc.vector.tensor_copy(warm_sink[:], wp[0:1, 0:1])

        # ---- per side: binarize -> hash matmuls -> sign -> agree -> plane.
        # The full k-side chain is emitted (and thus prioritized) before the
        # q-side: its one-hot feeds the critical scan, while the q-side plane
        # is only needed by the late gather. The hash runs as two fp16
        # matmuls (W-hi + W-lo) accumulating into f32 psum; chunk pairs share
        # a [64, 512] psum tile at partition bases {0, 32}, and the base-0
        # chunk uses the zero-padded M=40 weights so rows 8-31 are defined
        # for the batched Sign.
        from concourse.tile_rust import add_dep_helper  # noqa: E402

        NCH = L // 512
        onehot = sb.tile([128, L], dt.float16, tag="onehot")
        q1h = sb.tile([128, L], dt.float16, tag="q1h")
        relu_k = {}
        agree_k_last = {}
        for side, src_sb in (("k", kT_sb), ("q", qT_sb)):
            x = sb.tile([D, L], dt.float16, tag=f"x{side}")
            if side == "k":
                nc.vector.tensor_single_scalar(
                    x[:, 0:HALF], src_sb[:, 0:HALF], 0.0, Alu.is_gt
                )
                nc.vector.tensor_single_scalar(
                    x[:, HALF:L], src_sb[:, HALF:L], 0.0, Alu.is_gt
                )
            else:
                nc.vector.tensor_single_scalar(x[:], src_sb[:], 0.0, Alu.is_gt)
            dst = onehot if side == "k" else q1h
            for g in range(NCH // 2):
                t = ps.tile([64, 512], dt.float32, tag="hp")
                for u in range(2):
                    c = 2 * g + u
                    m = MPAD if u == 0 else NH
                    mm = nc.tensor.matmul(
                        t[32 * u : 32 * u + m, :],
                        lhsT=wpk_sb[:, 0:m], rhs=x[:, 512 * c : 512 * (c + 1)],
                        start=True, stop=False,
                    )
                    if side == "q" and g == 0 and u == 0 and 0 in agree_k_last:
                        add_dep_helper(
                            mm.ins, agree_k_last[0].ins, sync=False,
                            reason="k agree before q hash on PE",
                        )
                    nc.tensor.matmul(
                        t[32 * u : 32 * u + m, :],
                        lhsT=wpk_sb[:, MPAD : MPAD + m],
                        rhs=x[:, 512 * c : 512 * (c + 1)],
                        start=False, stop=True,
                    )
                s = sb.tile([MPAD, 512], dt.float16, tag=f"sgn{side}{g}")
                sign_inst = nc.scalar.activation(s[:], t[0:MPAD, :], Act.Sign)
                if side == "q" and g in relu_k:
                    add_dep_helper(
                        sign_inst.ins, relu_k[g].ins, sync=False,
                        reason="k one-hot before q signs on ACT",
                    )
                apt = aps.tile([128, 1024], dt.float32, tag="agree")
                for u in range(2):
                    agm = nc.tensor.matmul(
                        apt[:, 512 * u : 512 * (u + 1)],
                        lhsT=sgnc_sb[32 * u : 32 * u + 8, :],
                        rhs=s[32 * u : 32 * u + 8, :],
                        start=True, stop=True,
                    )
                if side == "k":
                    agree_k_last[g] = agm
                relu_inst = nc.scalar.activation(
                    dst[:, 1024 * g : 1024 * (g + 1)], apt[:],
                    Act.Relu, bias=bias7[:],
                )
                if side == "k":
                    relu_k[g] = relu_inst

        # ---- rank keys within bucket (inclusive prefix sum along j), in two
        # halves pipelined against the GPSIMD table scatters ----
        rank = sb.tile([128, L], dt.float16, tag="rank")
        m1 = sb.tile([128, L], dt.float16, tag="m1")
        idx16 = sb.tile([128, L], dt.int16, tag="idx16")
        tabs = []
        idx_inst = {}
        for h in range(2):
            lo, hi = HALF * h, HALF * (h + 1)
            init = 0.0 if h == 0 else rank[:, HALF - 1 : HALF]
            scan_inst = nc.vector.tensor_tensor_scan(
                rank[:, lo:hi], onehot[:, lo:hi], onehot[:, lo:hi],
                init, Alu.add, Alu.bypass,
            )
            if h == 1:
                add_dep_helper(
                    scan_inst.ins, idx_inst[0].ins, sync=False,
                    reason="finish half-0 scatter chain before scan half-1",
                )
            # m1 = onehot * rank (global rank at matches, 0 elsewhere);
            # scatter index = m1 - 1 (-1 = ignored by local_scatter)
            nc.vector.tensor_mul(m1[:, lo:hi], onehot[:, lo:hi], rank[:, lo:hi])
            idx_inst[h] = nc.vector.tensor_single_scalar(
                idx16[:, lo:hi], m1[:, lo:hi], 1.0, Alu.subtract
            )
            tab = sb.tile([128, TABLE_ELEMS], dt.int16, tag=f"table{h}")
            tabs.append(tab)
            nc.gpsimd.local_scatter(
                tab[:], iota_sb[:, lo:hi], idx16[:, lo:hi],
                channels=128, num_elems=TABLE_ELEMS, num_idxs=HALF,
            )
        # merge the two half-tables (disjoint nonzero slots) -> fp16
        tab16 = sb.tile([128, KMAX], dt.float16, tag="tab16")
        nc.vector.tensor_add(tab16[:], tabs[0][:, 0:KMAX], tabs[1][:, 0:KMAX])

        # ---- gather per query: out[i, s] = sum_c q1h[c, i] * tab16[c, s] ----
        # Chunk t takes queries {16p + t}, so psum partition p holds queries
        # 16p..16p+16 across chunks -> contiguous per-partition DRAM rows.
        q1h_v = q1h[:].rearrange("c (i t) -> c t i", t=16)
        op = aps.tile([128, 16 * KMAX], dt.float32, tag="agree")
        for t in range(16):
            nc.tensor.matmul(
                op[:, KMAX * t : KMAX * (t + 1)],
                lhsT=q1h_v[:, t, :], rhs=tab16[:],
                start=True, stop=True,
            )
        out_sb = sb.tile([128, 16 * KMAX], dt.float16, tag="out_sb")
        out_v = out_d[:].rearrange("(p t) s -> p (t s)", p=128)
        HO = 8 * KMAX
        for h in range(2):
            nc.scalar.activation(
                out_sb[:, HO * h : HO * (h + 1)], op[:, HO * h : HO * (h + 1)],
                Act.Copy,
            )
            nc.sync.dma_start(
                out_v[:, HO * h : HO * (h + 1)], out_sb[:, HO * h : HO * (h + 1)]
            )

    nc.compile()
    return nc


def _get_nc():
    if "nc" not in _cache:
        _cache["nc"] = _build_program()
    return _cache["nc"]


def _make_in_maps(query, key, W):
    query = np.asarray(query, dtype=np.float32)
    key = np.asarray(key, dtype=np.float32)
    W = np.asarray(W, dtype=np.float32)
    qT = [
        np.ascontiguousarray(query[b].T).astype(ml_dtypes.bfloat16) for b in range(B)
    ]
    kT = [np.ascontiguousarray(key[b].T).astype(ml_dtypes.bfloat16) for b in range(B)]

    wpk = np.zeros((D, 2 * MPAD), np.float16)
    wpk[:, :NH] = W.astype(np.float16)
    wpk[:, MPAD : MPAD + NH] = (W - wpk[:, :NH].astype(np.float32)).astype(np.float16)

    sgnc = []
    for h in range(2):
        cg = 128 * h + np.arange(128)  # global bucket ids of this half
        bits = ((cg[None, :] >> np.arange(NH)[:, None]) & 1).astype(np.float32)
        pm = (2.0 * bits - 1.0).astype(np.float16)  # [8, 128]
        arr = np.zeros((MPAD, 128), np.float16)
        arr[0:NH] = pm
        arr[32 : 32 + NH] = pm
        sgnc.append(arr)
    return [
        {"qT": qT[c // 2], "kT": kT[c // 2], "wpk": wpk, "sgnc": sgnc[c % 2]}
        for c in range(2 * B)
    ]


def _combine(results):
    out = np.empty((B, L, KMAX), dtype=np.int64)
    for b in range(B):
        g = results[2 * b]["out"].astype(np.int64) + results[2 * b + 1]["out"].astype(
            np.int64
        )
        out[b] = g - 1
    return out


def _run_spmd(in_maps, **kwargs):
    from concourse.bass_utils import run_bass_kernel_spmd

    return run_bass_kernel_spmd(_get_nc(), in_maps, list(range(2 * B)), **kwargs)


def kernel(query, key, W, head_idx=0, **_unused):
    in_maps = _make_in_maps(query, key, W)
    res = _run_spmd(in_maps)
    return _combine(res.results)

